# revision 34
# baseline (speedup 1.0000x reference)
"""CoordAttention kernel for Trainium2 (8 NeuronCores, data-parallel over batch).

fp16 data path: the 2e-2 rel-err budget is huge, so x is converted to fp16 on
the host. That halves HBM traffic (the roofline for this memory-bound problem)
and doubles/quadruples DVE throughput (2-byte dtypes enable the 2x/4x DVE
perf modes). PE identity-matmul sums run at 1 col/cycle in fp16.

Reference computation (per sample, inference):
  pools:  mean/max over W -> [C,H];  mean/max over H -> [C,W];  global -> [C]
  x_cat = concat(pools) -> [C, H+W+1, 2]
  y = BN(conv1x1(x_cat)) ; y = h_swish(y)
  a_h = sigmoid(conv(xh)), a_w = sigmoid(conv(xw)), a_c = sigmoid(conv(xc))
  out = x * a_w * a_h * a_c

Mapping onto one NeuronCore (2 samples each):
  - x stored [c(128-part) x (h, w)] fp16, one tile per 128-channel group
  - sum-pools on TensorE: fp16 identity-weight matmuls accumulating in PSUM
  - max-pools on VectorE: reduce_max (over w) + tensor_tensor max tree (over h)
  - conv1/BN folded host-side; small convs on TensorE; sigmoid/h_swish on
    ScalarE/VectorE
  - final multiply on VectorE: one broadcast multiply by a_w, then one
    broadcast multiply by s = a_h*a_c stored as duplicated pairs so the
    access pattern keeps a packed innermost dim (4x DVE mode)
"""
import sys

for _p in ("/opt/trn_rl_repo", "/root/.axon_site/_ro/trn_rl_repo"):
    if _p not in sys.path:
        sys.path.insert(0, _p)

import copy as _copy
import numpy as np

import concourse.bass as bass
import concourse.mybir as mybir
import concourse.tile as tile

f32 = mybir.dt.float32
f16 = mybir.dt.float16
OP = mybir.AluOpType
AF = mybir.ActivationFunctionType
AX = mybir.AxisListType

N, C, H, W = 16, 256, 128, 128
TC = 8
NCORES = 8
PER = N // NCORES
EPS = 1e-5
CT = C // 128           # channel tiles per sample
SS = H + W + 1          # pooled sequence length
JJ = 4                  # h-groups per sum-over-h matmul
J = 4                   # w-groups per sum-over-w matmul
G8 = 8                  # first-stage groups in the max trees


def _split_excess_waits(nc, limit=1):
    """This container's walrus accepts only one sync-wait per instruction;
    hoist extras onto same-engine drain carriers inserted just before."""
    m = nc.m
    newm = _copy.replace(m, functions=[])
    for fn in m.functions:
        newfn = _copy.replace(fn, blocks=[])
        newfn.set_allocations_from_list(fn.allocations)
        for blk in fn.blocks:
            out = []
            for inst in blk.instructions:
                si = inst.sync_info
                waits = list(si.on_wait) if si and si.on_wait else []
                if len(waits) > limit:
                    keep, excess = waits[-limit:], waits[: len(waits) - limit]
                    for gi, wchunk in enumerate(excess):
                        d = mybir.InstDrain(
                            name=f"{inst.name}-wsplit{gi}", ins=[], outs=[]
                        )
                        d.engine = inst.engine
                        d.sync_info = mybir.SyncInfo(on_wait=[wchunk], on_update=[])
                        out.append(d)
                    inst.sync_info = mybir.SyncInfo(
                        on_wait=keep, on_update=list(si.on_update or [])
                    )
                out.append(inst)
            newfn.blocks.append(_copy.replace(blk, instructions=out))
        newm.functions.append(newfn)
    nc.m = newm


def build_nc(per=PER, xp_bufs=5, split_waits=True):
    nc = bass.Bass()
    x_d = nc.declare_dram_parameter("x16", [per, C, H, W], f16, isOutput=False)
    o_d = nc.declare_dram_parameter("out", [per, C, H, W], f16, isOutput=True)
    id_d = nc.declare_dram_parameter("ident16", [128, 128], f16, isOutput=False)
    w1_d = nc.declare_dram_parameter("w1t", [C, TC], f16, isOutput=False)
    b1_d = nc.declare_dram_parameter("b1f", [TC, 1], f32, isOutput=False)
    w2_d = nc.declare_dram_parameter("w2t", [TC, 2, C], f16, isOutput=False)
    w3_d = nc.declare_dram_parameter("w3t", [TC, 2, C], f16, isOutput=False)
    w4_d = nc.declare_dram_parameter("w4t", [TC, 2, C], f16, isOutput=False)
    b2_d = nc.declare_dram_parameter("b2r", [C, 1], f32, isOutput=False)
    b3_d = nc.declare_dram_parameter("b3r", [C, 1], f32, isOutput=False)
    b4_d = nc.declare_dram_parameter("b4r", [C, 1], f32, isOutput=False)

    def tree_h(eng, ch, out, scr):
        """max over h: ch [128, H, W] -> out [128, W]; scr [128, H//G8, W]."""
        gsz = H // G8
        eng.tensor_tensor(
            out=scr, in0=ch[:, 0:gsz, :], in1=ch[:, gsz : 2 * gsz, :], op=OP.max
        )
        for i in range(2, G8):
            eng.tensor_tensor(
                out=scr, in0=scr, in1=ch[:, i * gsz : (i + 1) * gsz, :], op=OP.max
            )
        g = gsz
        while g > 2:
            eng.tensor_tensor(
                out=scr[:, 0 : g // 2, :],
                in0=scr[:, 0 : g // 2, :],
                in1=scr[:, g // 2 : g, :],
                op=OP.max,
            )
            g //= 2
        eng.tensor_tensor(out=out, in0=scr[:, 0, :], in1=scr[:, 1, :], op=OP.max)

    def tree_w_rows(eng, ch, out, scr, r0, r1):
        """max over w for rows r0:r1 on one engine: ch [128, H, W] ->
        out [128, r1-r0]; scr [128, r1-r0, W//G8]. Row ranges are
        independent, so the two halves need no cross-engine merge and can
        start as soon as their half of the DMA has landed."""
        gsz = W // G8
        eng.tensor_tensor(
            out=scr,
            in0=ch[:, r0:r1, 0:gsz],
            in1=ch[:, r0:r1, gsz : 2 * gsz],
            op=OP.max,
        )
        for i in range(2, G8):
            eng.tensor_tensor(
                out=scr, in0=scr, in1=ch[:, r0:r1, i * gsz : (i + 1) * gsz],
                op=OP.max,
            )
        g = gsz
        while g > 2:
            eng.tensor_tensor(
                out=scr[:, :, 0 : g // 2],
                in0=scr[:, :, 0 : g // 2],
                in1=scr[:, :, g // 2 : g],
                op=OP.max,
            )
            g //= 2
        eng.tensor_tensor(out=out, in0=scr[:, :, 0], in1=scr[:, :, 1], op=OP.max)

    with tile.TileContext(nc) as tc:
        with (
            tc.tile_pool(name="const", bufs=1) as cp,
            tc.tile_pool(name="xp", bufs=xp_bufs) as xp,
            tc.tile_pool(name="scp", bufs=2) as scp,
            tc.tile_pool(name="xcp", bufs=3) as xcp,
            tc.tile_pool(name="smp", bufs=2) as smp,
            tc.tile_pool(name="atp", bufs=3) as atp,
            tc.tile_pool(name="spool", bufs=2, space="PSUM") as spool,
            tc.tile_pool(name="apsum", bufs=2, space="PSUM") as apsum,
        ):
            # weights go through the ACT-engine DGE queue so the first x-tile
            # load starts immediately on the SP queue
            ident = cp.tile([128, 128], f16)
            nc.scalar.dma_start(out=ident, in_=id_d[:, :])
            w1sb = cp.tile([128, CT, TC], f16)
            nc.scalar.dma_start(
                out=w1sb, in_=w1_d.rearrange("(ct c) t -> c ct t", ct=CT)
            )
            b1sb = cp.tile([TC, 1], f32)
            nc.scalar.dma_start(out=b1sb, in_=b1_d[:, :])
            wsb = {}
            for nm, d in (("w2", w2_d), ("w3", w3_d), ("w4", w4_d)):
                t = cp.tile([TC, 2, C], f16, tag=f"wsb_{nm}")
                nc.scalar.dma_start(out=t, in_=d[:, :, :])
                wsb[nm] = t
            bsb = {}
            for nm, d in (("b2", b2_d), ("b3", b3_d), ("b4", b4_d)):
                t = cp.tile([128, CT, 1], f32, tag=f"bsb_{nm}")
                nc.scalar.dma_start(
                    out=t, in_=d.rearrange("(ct c) one -> c ct one", ct=CT)
                )
                bsb[nm] = t

            # PE observes the identity once, so later matmuls carry one wait.
            warm = apsum.tile([128, 128], f32, tag="ap")
            nc.tensor.matmul(warm, ident, ident, start=True, stop=True)

            HF = H // 2
            for s in range(per):
                # ---- loads (two row-halves per tile so trees/sums on the
                # first half start at half-transfer) ----
                xt = {}
                for ct in range(CT):
                    t = xp.tile([128, H, W], f16, tag="x")
                    for hh in range(2):
                        nc.sync.dma_start(
                            out=t[:, hh * HF : (hh + 1) * HF, :],
                            in_=x_d[
                                s, ct * 128 : (ct + 1) * 128,
                                hh * HF : (hh + 1) * HF, :,
                            ],
                        )
                    xt[ct] = t

                # ---- pools ----
                xcs = {}
                for ct in range(CT):
                    ch = xt[ct]
                    xcat = xcp.tile([128, 2, SS], f16, tag="xc")
                    # sum over h (PE): adjacent-row pairs per matmul so the
                    # first row-half of the DMA unblocks the front of the loop
                    psh = spool.tile([128, JJ, W], f32, tag="psh")
                    chv = ch.rearrange("p (g jj) w -> p jj g w", jj=JJ)
                    gmax = H // JJ
                    for g in range(gmax):
                        nc.tensor.matmul(
                            psh, ident, chv[:, :, g, :],
                            start=(g == 0), stop=(g == gmax - 1),
                        )
                    # sum over w (PE): adjacent-column pairs (needs full tile)
                    psw = spool.tile([128, J, H], f32, tag="psw")
                    cwv = ch.rearrange("p h (g j) -> p j g h", j=J)
                    gmax = W // J
                    for g in range(gmax):
                        nc.tensor.matmul(
                            psw, ident, cwv[:, :, g, :],
                            start=(g == 0), stop=(g == gmax - 1),
                        )
                    # max over w (DVE tree; walrus only allows TT on DVE)
                    scw = scp.tile([128, H, W // G8], f16, tag="scw")
                    tree_w_rows(nc.vector, ch, xcat[:, 1, 0:H], scw, 0, H)
                    # max over h (DVE tree)
                    sch = scp.tile([128, H // G8, W], f16, tag="sch")
                    tree_h(nc.vector, ch, xcat[:, 1, H : H + W], sch)
                    # global max
                    nc.vector.reduce_max(
                        out=xcat[:, 1, H + W : SS], in_=xcat[:, 1, 0:H], axis=AX.X
                    )
                    # fold the group-sums (DVE; walrus allows only one PSUM
                    # input per instruction, so copy one group out first)
                    sw = smp.tile([128, H], f32, tag="sw")
                    nc.vector.tensor_copy(out=sw, in_=psw[:, 0, :])
                    for j in range(1, J):
                        nc.vector.tensor_tensor(
                            out=sw, in0=sw, in1=psw[:, j, :], op=OP.add
                        )
                    sh = smp.tile([128, W], f32, tag="sh")
                    nc.vector.tensor_copy(out=sh, in_=psh[:, 0, :])
                    for j in range(1, JJ):
                        nc.vector.tensor_tensor(
                            out=sh, in0=sh, in1=psh[:, j, :], op=OP.add
                        )
                    # means (ACT): mean over w -> 0..H (+ global via accum)
                    acc = smp.tile([128, 1], f32, tag="acc")
                    nc.scalar.activation(
                        out=xcat[:, 0, 0:H], in_=sw, func=AF.Copy,
                        scale=1.0 / W, accum_out=acc,
                    )
                    nc.scalar.activation(
                        out=xcat[:, 0, H : H + W], in_=sh, func=AF.Copy,
                        scale=1.0 / H,
                    )
                    nc.scalar.activation(
                        out=xcat[:, 0, H + W : SS], in_=acc, func=AF.Copy,
                        scale=1.0 / H,
                    )
                    xcs[ct] = xcat

                # ---- conv1 + h_swish (tiny) ----
                xh = smp.tile([TC, 2, SS], f16, tag="xh")
                for k in range(2):
                    yp = apsum.tile([TC, SS], f32, tag="y")
                    for ct in range(CT):
                        nc.tensor.matmul(
                            yp, w1sb[:, ct, :], xcs[ct][:, k, :],
                            start=(ct == 0), stop=(ct == CT - 1),
                        )
                    xhk = xh[:, k, :]
                    nc.scalar.add(out=xhk, in_=yp, add=b1sb)
                    u = smp.tile([TC, SS], f16, tag="u")
                    nc.vector.tensor_scalar(
                        out=u, in0=xhk,
                        scalar1=-3.0, scalar2=3.0, op0=OP.max, op1=OP.min,
                    )
                    # xhk = (u + 3) * xhk in one fused DVE op
                    nc.vector.scalar_tensor_tensor(
                        out=xhk, in0=u, scalar=3.0, in1=xhk,
                        op0=OP.add, op1=OP.mult,
                    )

                # ---- attention maps + final scale ----
                for ct in range(CT):
                    att = atp.tile([128, SS], f16, tag="att")
                    ahp = apsum.tile([128, H], f32, tag="ap")
                    for k in range(2):
                        nc.tensor.matmul(
                            ahp, wsb["w2"][:, k, ct * 128 : (ct + 1) * 128],
                            xh[:, k, 0:H], start=(k == 0), stop=(k == 1),
                        )
                    nc.scalar.activation(
                        out=att[:, 0:H], in_=ahp, func=AF.Sigmoid,
                        bias=bsb["b2"][:, ct, :], scale=1.0,
                    )
                    awp = apsum.tile([128, W], f32, tag="ap")
                    for k in range(2):
                        nc.tensor.matmul(
                            awp, wsb["w3"][:, k, ct * 128 : (ct + 1) * 128],
                            xh[:, k, H : H + W], start=(k == 0), stop=(k == 1),
                        )
                    nc.scalar.activation(
                        out=att[:, H : H + W], in_=awp, func=AF.Sigmoid,
                        bias=bsb["b3"][:, ct, :], scale=1.0,
                    )
                    acp = apsum.tile([128, 1], f32, tag="ap")
                    for k in range(2):
                        nc.tensor.matmul(
                            acp, wsb["w4"][:, k, ct * 128 : (ct + 1) * 128],
                            xh[:, k, H + W : SS], start=(k == 0), stop=(k == 1),
                        )
                    ac = atp.tile([128, 1], f32, tag="ac")
                    nc.scalar.activation(
                        out=ac, in_=acp, func=AF.Sigmoid,
                        bias=bsb["b4"][:, ct, :], scale=1.0,
                    )
                    # s = a_h * a_c as duplicated f16 pairs: TensorTensor
                    # only has the 2x_1p perf mode (packed 2-byte innermost on
                    # every operand) - a [1,2] view of the pairs qualifies,
                    # a stride-0 broadcast does not
                    s2 = atp.tile([128, H, 2], f16, tag="s2")
                    for half in range(2):
                        nc.vector.tensor_scalar_mul(
                            out=s2[:, :, half], in0=att[:, 0:H], scalar1=ac
                        )
                    # apply (DVE): aw broadcast TT then s2-pair TT, in place,
                    # per row-half so each output DMA piece starts early
                    ch = xt[ct]
                    for hh in range(2):
                        chp = ch[:, hh * HF : (hh + 1) * HF, :]
                        awb = att[:, H : H + W].unsqueeze(1).to_broadcast(
                            [128, HF, W]
                        )
                        nc.vector.tensor_tensor(
                            out=chp, in0=chp, in1=awb, op=OP.mult
                        )
                        xv = chp.rearrange("p h (w1 w0) -> p h w1 w0", w0=2)
                        s2b = s2[:, hh * HF : (hh + 1) * HF, :].unsqueeze(
                            2
                        ).to_broadcast([128, HF, W // 2, 2])
                        nc.vector.tensor_tensor(
                            out=xv, in0=xv, in1=s2b, op=OP.mult
                        )
                        nc.sync.dma_start(
                            out=o_d[
                                s, ct * 128 : (ct + 1) * 128,
                                hh * HF : (hh + 1) * HF, :,
                            ],
                            in_=chp,
                        )

    if split_waits:
        _split_excess_waits(nc)
    return nc


def prep_weights(w1, b1, bn_gamma, bn_beta, bn_mean, bn_var, w2, b2, w3, b3, w4, b4):
    inv = (bn_gamma / np.sqrt(bn_var + EPS)).astype(np.float32)
    w1f = (w1 * inv[:, None]).astype(np.float32)          # [TC, C]
    b1f = ((b1 - bn_mean) * inv + bn_beta).astype(np.float32)
    def pack(wk):  # [C, TC, 2] -> [TC, 2, C], with the h_swish /6 folded in
        return np.ascontiguousarray(wk.transpose(1, 2, 0) / 6.0).astype(np.float16)
    return dict(
        ident16=np.eye(128, dtype=np.float16),
        w1t=np.ascontiguousarray(w1f.T).astype(np.float16),   # [C, TC]
        b1f=b1f.reshape(TC, 1),
        w2t=pack(w2), w3t=pack(w3), w4t=pack(w4),
        b2r=b2.reshape(C, 1).astype(np.float32),
        b3r=b3.reshape(C, 1).astype(np.float32),
        b4r=b4.reshape(C, 1).astype(np.float32),
    )


_NC_CACHE = {}


def _get_nc():
    if "nc" not in _NC_CACHE:
        _NC_CACHE["nc"] = build_nc()
    return _NC_CACHE["nc"]


def make_in_maps(x, w1, b1, bn_gamma, bn_beta, bn_mean, bn_var,
                 w2, b2, w3, b3, w4, b4):
    x16 = np.asarray(x).astype(np.float16)
    wmap = prep_weights(
        np.asarray(w1, np.float32), np.asarray(b1, np.float32),
        np.asarray(bn_gamma, np.float32), np.asarray(bn_beta, np.float32),
        np.asarray(bn_mean, np.float32), np.asarray(bn_var, np.float32),
        np.asarray(w2, np.float32), np.asarray(b2, np.float32),
        np.asarray(w3, np.float32), np.asarray(b3, np.float32),
        np.asarray(w4, np.float32), np.asarray(b4, np.float32),
    )
    return [
        {"x16": np.ascontiguousarray(x16[i * PER : (i + 1) * PER]), **wmap}
        for i in range(NCORES)
    ]


def gather_out(results):
    return np.concatenate(
        [results[i]["out"] for i in range(NCORES)], axis=0
    ).astype(np.float32)


def kernel(x, w1, b1, bn_gamma, bn_beta, bn_mean, bn_var, w2, b2, w3, b3, w4, b4):
    from concourse.bass_utils import run_bass_kernel_spmd

    nc = _get_nc()
    in_maps = make_in_maps(x, w1, b1, bn_gamma, bn_beta, bn_mean, bn_var,
                           w2, b2, w3, b3, w4, b4)
    res = run_bass_kernel_spmd(nc, in_maps, core_ids=list(range(NCORES)))
    return gather_out(res.results)


# revision 35
# speedup vs baseline: 1.0658x; 1.0658x over previous
"""CoordAttention kernel for Trainium2 (8 NeuronCores, data-parallel over batch).

fp16 data path: the 2e-2 rel-err budget is huge, so x is converted to fp16 on
the host. That halves HBM traffic (the roofline for this memory-bound problem)
and doubles/quadruples DVE throughput (2-byte dtypes enable the 2x/4x DVE
perf modes). PE identity-matmul sums run at 1 col/cycle in fp16.

Reference computation (per sample, inference):
  pools:  mean/max over W -> [C,H];  mean/max over H -> [C,W];  global -> [C]
  x_cat = concat(pools) -> [C, H+W+1, 2]
  y = BN(conv1x1(x_cat)) ; y = h_swish(y)
  a_h = sigmoid(conv(xh)), a_w = sigmoid(conv(xw)), a_c = sigmoid(conv(xc))
  out = x * a_w * a_h * a_c

Mapping onto one NeuronCore (2 samples each):
  - x stored [c(128-part) x (h, w)] fp16, one tile per 128-channel group
  - sum-pools on TensorE: fp16 identity-weight matmuls accumulating in PSUM
  - max-pools on VectorE: reduce_max (over w) + tensor_tensor max tree (over h)
  - conv1/BN folded host-side; small convs on TensorE; sigmoid/h_swish on
    ScalarE/VectorE
  - final multiply on VectorE: one broadcast multiply by a_w, then one
    broadcast multiply by s = a_h*a_c stored as duplicated pairs so the
    access pattern keeps a packed innermost dim (4x DVE mode)
"""
import sys

for _p in ("/opt/trn_rl_repo", "/root/.axon_site/_ro/trn_rl_repo"):
    if _p not in sys.path:
        sys.path.insert(0, _p)

import copy as _copy
import numpy as np

import concourse.bass as bass
import concourse.mybir as mybir
import concourse.tile as tile

f32 = mybir.dt.float32
f16 = mybir.dt.float16
OP = mybir.AluOpType
AF = mybir.ActivationFunctionType
AX = mybir.AxisListType

N, C, H, W = 16, 256, 128, 128
TC = 8
NCORES = 8
PER = N // NCORES
EPS = 1e-5
CT = C // 128           # channel tiles per sample
SS = H + W + 1          # pooled sequence length
JJ = 2                  # h-groups per sum-over-h matmul
J = 2                   # w-groups per sum-over-w matmul
G8 = 8                  # first-stage groups in the max trees


def _split_excess_waits(nc, limit=1):
    """This container's walrus accepts only one sync-wait per instruction;
    hoist extras onto same-engine drain carriers inserted just before."""
    m = nc.m
    newm = _copy.replace(m, functions=[])
    for fn in m.functions:
        newfn = _copy.replace(fn, blocks=[])
        newfn.set_allocations_from_list(fn.allocations)
        for blk in fn.blocks:
            out = []
            for inst in blk.instructions:
                si = inst.sync_info
                waits = list(si.on_wait) if si and si.on_wait else []
                if len(waits) > limit:
                    keep, excess = waits[-limit:], waits[: len(waits) - limit]
                    for gi, wchunk in enumerate(excess):
                        d = mybir.InstDrain(
                            name=f"{inst.name}-wsplit{gi}", ins=[], outs=[]
                        )
                        d.engine = inst.engine
                        d.sync_info = mybir.SyncInfo(on_wait=[wchunk], on_update=[])
                        out.append(d)
                    inst.sync_info = mybir.SyncInfo(
                        on_wait=keep, on_update=list(si.on_update or [])
                    )
                out.append(inst)
            newfn.blocks.append(_copy.replace(blk, instructions=out))
        newm.functions.append(newfn)
    nc.m = newm


def build_nc(per=PER, xp_bufs=5, split_waits=True):
    nc = bass.Bass()
    x_d = nc.declare_dram_parameter("x16", [per, C, H, W], f16, isOutput=False)
    o_d = nc.declare_dram_parameter("out", [per, C, H, W], f16, isOutput=True)
    id_d = nc.declare_dram_parameter("ident16", [128, 128], f16, isOutput=False)
    w1_d = nc.declare_dram_parameter("w1t", [C, TC], f16, isOutput=False)
    b1_d = nc.declare_dram_parameter("b1f", [TC, 1], f32, isOutput=False)
    w2_d = nc.declare_dram_parameter("w2t", [TC, 2, C], f16, isOutput=False)
    w3_d = nc.declare_dram_parameter("w3t", [TC, 2, C], f16, isOutput=False)
    w4_d = nc.declare_dram_parameter("w4t", [TC, 2, C], f16, isOutput=False)
    b2_d = nc.declare_dram_parameter("b2r", [C, 1], f32, isOutput=False)
    b3_d = nc.declare_dram_parameter("b3r", [C, 1], f32, isOutput=False)
    b4_d = nc.declare_dram_parameter("b4r", [C, 1], f32, isOutput=False)

    def tree_h(eng, ch, out, scr):
        """max over h: ch [128, H, W] -> out [128, W]; scr [128, H//G8, W]."""
        gsz = H // G8
        eng.tensor_tensor(
            out=scr, in0=ch[:, 0:gsz, :], in1=ch[:, gsz : 2 * gsz, :], op=OP.max
        )
        for i in range(2, G8):
            eng.tensor_tensor(
                out=scr, in0=scr, in1=ch[:, i * gsz : (i + 1) * gsz, :], op=OP.max
            )
        g = gsz
        while g > 2:
            eng.tensor_tensor(
                out=scr[:, 0 : g // 2, :],
                in0=scr[:, 0 : g // 2, :],
                in1=scr[:, g // 2 : g, :],
                op=OP.max,
            )
            g //= 2
        eng.tensor_tensor(out=out, in0=scr[:, 0, :], in1=scr[:, 1, :], op=OP.max)

    def tree_w_rows(eng, ch, out, scr, r0, r1):
        """max over w for rows r0:r1 on one engine: ch [128, H, W] ->
        out [128, r1-r0]; scr [128, r1-r0, W//G8]. Row ranges are
        independent, so the two halves need no cross-engine merge and can
        start as soon as their half of the DMA has landed."""
        gsz = W // G8
        eng.tensor_tensor(
            out=scr,
            in0=ch[:, r0:r1, 0:gsz],
            in1=ch[:, r0:r1, gsz : 2 * gsz],
            op=OP.max,
        )
        for i in range(2, G8):
            eng.tensor_tensor(
                out=scr, in0=scr, in1=ch[:, r0:r1, i * gsz : (i + 1) * gsz],
                op=OP.max,
            )
        g = gsz
        while g > 2:
            eng.tensor_tensor(
                out=scr[:, :, 0 : g // 2],
                in0=scr[:, :, 0 : g // 2],
                in1=scr[:, :, g // 2 : g],
                op=OP.max,
            )
            g //= 2
        eng.tensor_tensor(out=out, in0=scr[:, :, 0], in1=scr[:, :, 1], op=OP.max)

    with tile.TileContext(nc) as tc:
        with (
            tc.tile_pool(name="const", bufs=1) as cp,
            tc.tile_pool(name="xp", bufs=xp_bufs) as xp,
            tc.tile_pool(name="scp", bufs=2) as scp,
            tc.tile_pool(name="xcp", bufs=3) as xcp,
            tc.tile_pool(name="smp", bufs=2) as smp,
            tc.tile_pool(name="atp", bufs=3) as atp,
            tc.tile_pool(name="spool", bufs=2, space="PSUM") as spool,
            tc.tile_pool(name="apsum", bufs=2, space="PSUM") as apsum,
        ):
            # weights go through the ACT-engine DGE queue so the first x-tile
            # load starts immediately on the SP queue
            ident = cp.tile([128, 128], f16)
            nc.scalar.dma_start(out=ident, in_=id_d[:, :])
            w1sb = cp.tile([128, CT, TC], f16)
            nc.scalar.dma_start(
                out=w1sb, in_=w1_d.rearrange("(ct c) t -> c ct t", ct=CT)
            )
            b1sb = cp.tile([TC, 1], f32)
            nc.scalar.dma_start(out=b1sb, in_=b1_d[:, :])
            wsb = {}
            for nm, d in (("w2", w2_d), ("w3", w3_d), ("w4", w4_d)):
                t = cp.tile([TC, 2, C], f16, tag=f"wsb_{nm}")
                nc.scalar.dma_start(out=t, in_=d[:, :, :])
                wsb[nm] = t
            bsb = {}
            for nm, d in (("b2", b2_d), ("b3", b3_d), ("b4", b4_d)):
                t = cp.tile([128, CT, 1], f32, tag=f"bsb_{nm}")
                nc.scalar.dma_start(
                    out=t, in_=d.rearrange("(ct c) one -> c ct one", ct=CT)
                )
                bsb[nm] = t

            # PE observes the identity once, so later matmuls carry one wait.
            warm = apsum.tile([128, 128], f32, tag="ap")
            nc.tensor.matmul(warm, ident, ident, start=True, stop=True)

            HF = H // 2
            for s in range(per):
                # ---- loads (two row-halves per tile so trees/sums on the
                # first half start at half-transfer) ----
                xt = {}
                for ct in range(CT):
                    t = xp.tile([128, H, W], f16, tag="x")
                    for hh in range(2):
                        nc.sync.dma_start(
                            out=t[:, hh * HF : (hh + 1) * HF, :],
                            in_=x_d[
                                s, ct * 128 : (ct + 1) * 128,
                                hh * HF : (hh + 1) * HF, :,
                            ],
                        )
                    xt[ct] = t

                # ---- pools ----
                xcs = {}
                for ct in range(CT):
                    ch = xt[ct]
                    xcat = xcp.tile([128, 2, SS], f16, tag="xc")
                    # sum over h (PE): adjacent-row pairs per matmul so the
                    # first row-half of the DMA unblocks the front of the loop
                    psh = spool.tile([128, JJ, W], f32, tag="psh")
                    chv = ch.rearrange("p (g jj) w -> p jj g w", jj=JJ)
                    gmax = H // JJ
                    for g in range(gmax):
                        nc.tensor.matmul(
                            psh, ident, chv[:, :, g, :],
                            start=(g == 0), stop=(g == gmax - 1),
                        )
                    # sum over w (PE): adjacent-column pairs (needs full tile)
                    psw = spool.tile([128, J, H], f32, tag="psw")
                    cwv = ch.rearrange("p h (g j) -> p j g h", j=J)
                    gmax = W // J
                    for g in range(gmax):
                        nc.tensor.matmul(
                            psw, ident, cwv[:, :, g, :],
                            start=(g == 0), stop=(g == gmax - 1),
                        )
                    # max over w (DVE tree; walrus only allows TT on DVE)
                    scw = scp.tile([128, H, W // G8], f16, tag="scw")
                    tree_w_rows(nc.vector, ch, xcat[:, 1, 0:H], scw, 0, H)
                    # max over h (DVE tree)
                    sch = scp.tile([128, H // G8, W], f16, tag="sch")
                    tree_h(nc.vector, ch, xcat[:, 1, H : H + W], sch)
                    # global max
                    nc.vector.reduce_max(
                        out=xcat[:, 1, H + W : SS], in_=xcat[:, 1, 0:H], axis=AX.X
                    )
                    # fold the group-sums (DVE; walrus allows only one PSUM
                    # input per instruction, so copy one group out first)
                    sw = smp.tile([128, H], f32, tag="sw")
                    nc.vector.tensor_copy(out=sw, in_=psw[:, 0, :])
                    for j in range(1, J):
                        nc.vector.tensor_tensor(
                            out=sw, in0=sw, in1=psw[:, j, :], op=OP.add
                        )
                    sh = smp.tile([128, W], f32, tag="sh")
                    nc.vector.tensor_copy(out=sh, in_=psh[:, 0, :])
                    for j in range(1, JJ):
                        nc.vector.tensor_tensor(
                            out=sh, in0=sh, in1=psh[:, j, :], op=OP.add
                        )
                    # means (ACT): mean over w -> 0..H (+ global via accum)
                    acc = smp.tile([128, 1], f32, tag="acc")
                    nc.scalar.activation(
                        out=xcat[:, 0, 0:H], in_=sw, func=AF.Copy,
                        scale=1.0 / W, accum_out=acc,
                    )
                    nc.scalar.activation(
                        out=xcat[:, 0, H : H + W], in_=sh, func=AF.Copy,
                        scale=1.0 / H,
                    )
                    nc.scalar.activation(
                        out=xcat[:, 0, H + W : SS], in_=acc, func=AF.Copy,
                        scale=1.0 / H,
                    )
                    xcs[ct] = xcat

                # ---- conv1 + h_swish (tiny) ----
                xh = smp.tile([TC, 2, SS], f16, tag="xh")
                for k in range(2):
                    yp = apsum.tile([TC, SS], f32, tag="y")
                    for ct in range(CT):
                        nc.tensor.matmul(
                            yp, w1sb[:, ct, :], xcs[ct][:, k, :],
                            start=(ct == 0), stop=(ct == CT - 1),
                        )
                    xhk = xh[:, k, :]
                    nc.scalar.add(out=xhk, in_=yp, add=b1sb)
                    u = smp.tile([TC, SS], f16, tag="u")
                    nc.vector.tensor_scalar(
                        out=u, in0=xhk,
                        scalar1=-3.0, scalar2=3.0, op0=OP.max, op1=OP.min,
                    )
                    # xhk = (u + 3) * xhk in one fused DVE op
                    nc.vector.scalar_tensor_tensor(
                        out=xhk, in0=u, scalar=3.0, in1=xhk,
                        op0=OP.add, op1=OP.mult,
                    )

                # ---- attention maps + final scale ----
                for ct in range(CT):
                    att = atp.tile([128, SS], f16, tag="att")
                    ahp = apsum.tile([128, H], f32, tag="ap")
                    for k in range(2):
                        nc.tensor.matmul(
                            ahp, wsb["w2"][:, k, ct * 128 : (ct + 1) * 128],
                            xh[:, k, 0:H], start=(k == 0), stop=(k == 1),
                        )
                    nc.scalar.activation(
                        out=att[:, 0:H], in_=ahp, func=AF.Sigmoid,
                        bias=bsb["b2"][:, ct, :], scale=1.0,
                    )
                    awp = apsum.tile([128, W], f32, tag="ap")
                    for k in range(2):
                        nc.tensor.matmul(
                            awp, wsb["w3"][:, k, ct * 128 : (ct + 1) * 128],
                            xh[:, k, H : H + W], start=(k == 0), stop=(k == 1),
                        )
                    nc.scalar.activation(
                        out=att[:, H : H + W], in_=awp, func=AF.Sigmoid,
                        bias=bsb["b3"][:, ct, :], scale=1.0,
                    )
                    acp = apsum.tile([128, 1], f32, tag="ap")
                    for k in range(2):
                        nc.tensor.matmul(
                            acp, wsb["w4"][:, k, ct * 128 : (ct + 1) * 128],
                            xh[:, k, H + W : SS], start=(k == 0), stop=(k == 1),
                        )
                    ac = atp.tile([128, 1], f32, tag="ac")
                    nc.scalar.activation(
                        out=ac, in_=acp, func=AF.Sigmoid,
                        bias=bsb["b4"][:, ct, :], scale=1.0,
                    )
                    # s = a_h * a_c as duplicated f16 pairs: TensorTensor
                    # only has the 2x_1p perf mode (packed 2-byte innermost on
                    # every operand) - a [1,2] view of the pairs qualifies,
                    # a stride-0 broadcast does not
                    unit = s * CT + ct
                    s2 = atp.tile([128, H, 2], f16, tag="s2")
                    for half in range(2):
                        nc.vector.tensor_scalar_mul(
                            out=s2[:, :, half], in0=att[:, 0:H], scalar1=ac
                        )
                    if unit < 3:
                        s32 = atp.tile([128, 32], f32, tag="s32")
                        nc.scalar.activation(
                            out=s32, in_=att[:, H - 32 : H], func=AF.Copy,
                            scale=ac,
                        )
                    # apply (DVE): aw broadcast TT then s2-pair TT, in place,
                    # per row-half so each output DMA piece starts early
                    ch = xt[ct]
                    for hh in range(2):
                        chp = ch[:, hh * HF : (hh + 1) * HF, :]
                        awb = att[:, H : H + W].unsqueeze(1).to_broadcast(
                            [128, HF, W]
                        )
                        nc.vector.tensor_tensor(
                            out=chp, in0=chp, in1=awb, op=OP.mult
                        )
                        # s multiply: last 32 rows of early units go per-row
                        # on the Scalar engine (per-partition scale), the rest
                        # via the DVE s2-pair TT
                        nd = HF if not (unit < 3 and hh == 1) else HF - 32
                        if nd:
                            chd = ch[:, hh * HF : hh * HF + nd, :]
                            xv = chd.rearrange("p h (w1 w0) -> p h w1 w0", w0=2)
                            s2b = s2[:, hh * HF : hh * HF + nd, :].unsqueeze(
                                2
                            ).to_broadcast([128, nd, W // 2, 2])
                            nc.vector.tensor_tensor(
                                out=xv, in0=xv, in1=s2b, op=OP.mult
                            )
                        for r in range(hh * HF + nd, (hh + 1) * HF):
                            row = ch[:, r, :]
                            nc.scalar.activation(
                                out=row, in_=row, func=AF.Copy,
                                scale=s32[:, r - (H - 32) : r - (H - 32) + 1],
                            )
                        nc.sync.dma_start(
                            out=o_d[
                                s, ct * 128 : (ct + 1) * 128,
                                hh * HF : (hh + 1) * HF, :,
                            ],
                            in_=chp,
                        )

    if split_waits:
        _split_excess_waits(nc)
    return nc


def prep_weights(w1, b1, bn_gamma, bn_beta, bn_mean, bn_var, w2, b2, w3, b3, w4, b4):
    inv = (bn_gamma / np.sqrt(bn_var + EPS)).astype(np.float32)
    w1f = (w1 * inv[:, None]).astype(np.float32)          # [TC, C]
    b1f = ((b1 - bn_mean) * inv + bn_beta).astype(np.float32)
    def pack(wk):  # [C, TC, 2] -> [TC, 2, C], with the h_swish /6 folded in
        return np.ascontiguousarray(wk.transpose(1, 2, 0) / 6.0).astype(np.float16)
    return dict(
        ident16=np.eye(128, dtype=np.float16),
        w1t=np.ascontiguousarray(w1f.T).astype(np.float16),   # [C, TC]
        b1f=b1f.reshape(TC, 1),
        w2t=pack(w2), w3t=pack(w3), w4t=pack(w4),
        b2r=b2.reshape(C, 1).astype(np.float32),
        b3r=b3.reshape(C, 1).astype(np.float32),
        b4r=b4.reshape(C, 1).astype(np.float32),
    )


_NC_CACHE = {}


def _get_nc():
    if "nc" not in _NC_CACHE:
        _NC_CACHE["nc"] = build_nc()
    return _NC_CACHE["nc"]


def make_in_maps(x, w1, b1, bn_gamma, bn_beta, bn_mean, bn_var,
                 w2, b2, w3, b3, w4, b4):
    x16 = np.asarray(x).astype(np.float16)
    wmap = prep_weights(
        np.asarray(w1, np.float32), np.asarray(b1, np.float32),
        np.asarray(bn_gamma, np.float32), np.asarray(bn_beta, np.float32),
        np.asarray(bn_mean, np.float32), np.asarray(bn_var, np.float32),
        np.asarray(w2, np.float32), np.asarray(b2, np.float32),
        np.asarray(w3, np.float32), np.asarray(b3, np.float32),
        np.asarray(w4, np.float32), np.asarray(b4, np.float32),
    )
    return [
        {"x16": np.ascontiguousarray(x16[i * PER : (i + 1) * PER]), **wmap}
        for i in range(NCORES)
    ]


def gather_out(results):
    return np.concatenate(
        [results[i]["out"] for i in range(NCORES)], axis=0
    ).astype(np.float32)


def kernel(x, w1, b1, bn_gamma, bn_beta, bn_mean, bn_var, w2, b2, w3, b3, w4, b4):
    from concourse.bass_utils import run_bass_kernel_spmd

    nc = _get_nc()
    in_maps = make_in_maps(x, w1, b1, bn_gamma, bn_beta, bn_mean, bn_var,
                           w2, b2, w3, b3, w4, b4)
    res = run_bass_kernel_spmd(nc, in_maps, core_ids=list(range(NCORES)))
    return gather_out(res.results)
